# revision 20
# baseline (speedup 1.0000x reference)
"""Trainium2 Bass kernel for nn_CRFModel (PAC-CRF mean-field, 5 steps).

Sharding: 8 cores = batch (2) x h-stripe (4). Full-res softmax/update are
pointwise per stripe; the blur-res pooled softmax V is AllGather'd within
each 4-core batch group every step; the 11x11 pixel-adaptive conv runs as 11
PSUM-accumulated banded matmuls (w-band x h-shift) on a linearized RGB
kernel:  K0 ~= G_spatial * (c0 - c1*||dr||^2/2)  (minimax linear, err<=5e-6).
Kernel 1 is position-only at blur res => exact fixed separable Gaussian.
Bilinear upsample, 4x4 pooling and compat are fp32 PE matmuls.
"""
import numpy as np

def _bf16(x):
    import ml_dtypes
    return np.asarray(x, dtype=np.float32).astype(ml_dtypes.bfloat16)

C = 16; B = 2; H = W = 512; KS = 11; PAD = 5; NUM_STEPS = 5
UNARY_W = 0.8; PW0, PW1 = 2.0, 0.6; RGB_SCALE = 13.0
hb = H // 4; wb = W // 4                 # 128, 128
SH = 128                                 # full-res stripe rows
SB = 32                                  # blur-res stripe rows
NH = 44                                  # blur rows per core (34 out + 10)
NO = 34                                  # blur out rows (32 + 2 bilinear halo)
ZMAX = 3.0 * (1.0 / RGB_SCALE) ** 2 / 2.0
_c1 = (1.0 - np.exp(-ZMAX)) / ZMAX
_zs = -np.log(_c1)
_E = (1.0 - _c1 * _zs - np.exp(-_zs)) / 2.0
C0 = np.float32(1.0 - _E)
C1 = np.float32(_c1)

_CACHE = {}


def _host_consts():
    d = np.arange(-PAD, PAD + 1, dtype=np.float64)
    g0 = np.exp(-(d ** 2) / 800.0)
    g1 = np.exp(-8.0 * (d ** 2) / 9.0)

    def band(g):
        M = np.zeros((wb, wb), np.float32)
        for j in range(wb):
            for k in range(KS):
                i = j + k - PAD
                if 0 <= i < wb:
                    M[i, j] = np.float32(g[k])
        return M

    Gd0 = np.stack([np.float32(g0[k]) * band(g0) for k in range(KS)])
    Gd1 = np.stack([np.float32(g1[k]) * band(g1) for k in range(KS)])

    P4s = np.zeros((SH, SB), np.float32)
    for r in range(SH):
        P4s[r, r // 4] = 1.0 / 16.0

    def up_matrix(n_out, n_in):
        U = np.zeros((n_in, n_out), np.float32)
        s = n_in / n_out
        for r in range(n_out):
            y = (r + 0.5) * s - 0.5
            y0 = int(np.floor(y)); fr = np.float32(y - y0)
            U[min(max(y0, 0), n_in - 1), r] += np.float32(1) - fr
            U[min(max(y0 + 1, 0), n_in - 1), r] += fr
        return U

    Uw = up_matrix(W, wb)
    Uh_full = up_matrix(H, hb)
    Uh_loc = np.zeros((4, NO, SH), np.float32)
    for q in range(4):
        blk = Uh_full[:, SH * q: SH * (q + 1)]
        for i in range(NO):
            k = 32 * q - 1 + i
            if 0 <= k < hb:
                Uh_loc[q, i] = blk[k]
    P4i = np.zeros((92, 23), np.float32)
    for r in range(92):
        P4i[r, r // 4] = 1.0 / 16.0
    return dict(Gd0=Gd0, Gd1=Gd1, P4s=P4s, Uw=np.ascontiguousarray(Uw),
                Uh_loc=Uh_loc, P4i=P4i)


def _build():
    import concourse.bass as bass
    import concourse.bacc as bacc
    import concourse.tile as tile
    from concourse import mybir
    from contextlib import ExitStack

    f32 = mybir.dt.float32
    bf16 = mybir.dt.bfloat16
    AL = mybir.AluOpType
    ACTF = mybir.ActivationFunctionType
    X = mybir.AxisListType.X

    nc = bacc.Bacc("TRN2", target_bir_lowering=False, debug=False, num_devices=8)
    xs_d = nc.dram_tensor("xs", [C, SH, W], f32, kind="ExternalInput")
    img_d = nc.dram_tensor("imge", [3, 184, W], f32, kind="ExternalInput")
    uh_d = nc.dram_tensor("uh", [NO, SH], bf16, kind="ExternalInput")
    uhf_d = nc.dram_tensor("uhf", [NO, SH], f32, kind="ExternalInput")
    w0_d = nc.dram_tensor("w0r", [16, 16], f32, kind="ExternalInput")
    w1_d = nc.dram_tensor("w1r", [16, 16], f32, kind="ExternalInput")
    gd0_d = nc.dram_tensor("gd0", [KS, wb, wb], f32, kind="ExternalInput")
    gd1_d = nc.dram_tensor("gd1", [KS, wb, wb], f32, kind="ExternalInput")
    p4s_d = nc.dram_tensor("p4s", [SH, SB], f32, kind="ExternalInput")
    p4i_d = nc.dram_tensor("p4i", [92, 23], f32, kind="ExternalInput")
    uw_d = nc.dram_tensor("uw", [wb, W], f32, kind="ExternalInput")
    out_d = nc.dram_tensor("out", [C, SH, W], f32, kind="ExternalOutput")

    def bc(ap, n, at=1):
        """insert broadcast dim (step0 x n) at free position `at`."""
        dims = list(ap.ap)
        dims.insert(at, [0, n])
        return bass.AP(tensor=ap.tensor, offset=ap.offset, ap=dims)

    with tile.TileContext(nc) as tc, ExitStack() as ctx:
        sb = ctx.enter_context(tc.tile_pool(name="sb", bufs=1))
        sc = ctx.enter_context(tc.tile_pool(name="sc", bufs=1))
        dr = ctx.enter_context(tc.tile_pool(name="dr", bufs=1, space="DRAM"))

        q32 = nc.sync.partition_id() % 4 * 32

        logq = sb.tile([SH, C, W], f32)
        u08m = sb.tile([SH, C, W], f32)
        gd0 = sb.tile([wb, KS, wb], f32)
        nc.sync.dma_start(out=gd0[:], in_=gd0_d.ap().rearrange("k v w -> v k w"))
        gd1 = sb.tile([wb, KS, wb], f32)
        nc.sync.dma_start(out=gd1[:], in_=gd1_d.ap().rearrange("k v w -> v k w"))
        p4s = sb.tile([SH, SB], f32); nc.sync.dma_start(out=p4s[:], in_=p4s_d.ap())
        uw = sb.tile([wb, W], f32); nc.sync.dma_start(out=uw[:], in_=uw_d.ap())
        uwb = sb.tile([wb, W], bf16)
        msgb = sb.tile([wb, C, NO], bf16)       # post-min msg residual, bf16
        uhl = sb.tile([NO, SH], bf16); nc.sync.dma_start(out=uhl[:], in_=uh_d.ap())
        uhlf = sb.tile([NO, SH], f32); nc.sync.dma_start(out=uhlf[:], in_=uhf_d.ap())
        xwb = sb.tile([wb, C, wb], bf16)        # Up_w(msg-tmin) bf16, h padded to 128
        xtb = sb.tile([wb, 4, wb, C], bf16)     # xbar out: [h(34 valid), j, w, c]
        tcb = sb.tile([wb, wb], bf16)           # Up_w(tmin) bf16, h padded
        w01 = sb.tile([16, 32], f32)
        nc.sync.dma_start(out=w01[:, 0:16], in_=w0_d.ap())
        nc.sync.dma_start(out=w01[:, 16:32], in_=w1_d.ap())
        vcc = sb.tile([16, NH, wb], f32)        # gathered V, C-part
        rT = sb.tile([wb, 3, 46], f32)
        rhoT = sb.tile([wb, 46], f32)
        phi0 = sb.tile([wb, 46], f32)
        Dsum = sb.tile([SH, W], f32)
        Rrec = sb.tile([SH, W], f32)
        t8 = sb.tile([SH, 8, W], f32)
        t4 = sb.tile([SH, 4, W], f32)
        t2 = sb.tile([SH, 2, W], f32)

        vbounce = dr.tile([SB, C, wb], f32)
        gpad = dr.tile([140, C, wb], f32)
        v0d = dr.tile([C, NH, wb], f32)
        v1d = dr.tile([C, NH, wb], f32)
        xwd = dr.tile([4, wb, C, wb], bf16)
        twd = dr.tile([4, wb, wb], bf16)

        # ---------- init ----------
        with tc.tile_pool(name="ini", bufs=1) as ini:
            zpad = ini.tile([96, wb], f32)
            nc.vector.memset(zpad[:], 0.0)
            nc.sync.dma_start(out=gpad[:][0:6].rearrange("a b w -> (a b) w"), in_=zpad[:])
            nc.sync.dma_start(out=gpad[:][134:140].rearrange("a b w -> (a b) w"), in_=zpad[:])
            nc.vector.memset(xwb[:], 0.0)
            nc.vector.memset(tcb[:], 0.0)
            nc.vector.tensor_copy(uwb[:], uw[:])

            p4i = ini.tile([92, 23], f32)
            nc.sync.dma_start(out=p4i[:], in_=p4i_d.ap())
            for ch in range(2):
                imgc = ini.tile([92, 3, W], f32, tag="imgc")
                nc.sync.dma_start(
                    out=imgc[:],
                    in_=img_d.ap()[:, 92 * ch:92 * (ch + 1), :].rearrange("c h w -> h c w"))
                pw_ = ini.tile([92, 3, wb], f32, tag="pw_")
                nc.vector.reduce_sum(
                    out=pw_[:], in_=imgc[:].rearrange("p c (v k) -> p c v k", k=4), axis=X)
                with tc.tile_pool(name="psi", bufs=1, space="PSUM") as psi:
                    ip = psi.tile([23, 3, wb], f32, tag="ip")
                    nc.tensor.matmul(ip[:], p4i[:], pw_[:], start=True, stop=True)
                    ib = dr.tile([23, 3, wb], f32, tag="ib")
                    icp = ini.tile([23, 3, wb], f32, tag="icp")
                    nc.vector.tensor_copy(icp[:], ip[:])
                    nc.sync.dma_start(out=ib[:], in_=icp[:])
                for m3 in range(3):
                    nc.sync.dma_start(out=rT[:, m3, 23 * ch:23 * (ch + 1)],
                                      in_=ib[:][:, m3, :].rearrange("h w -> w h"))
            tmp3 = ini.tile([wb, 3, 46], f32)
            nc.vector.tensor_tensor(out=tmp3[:], in0=rT[:], in1=rT[:], op=AL.mult)
            nc.vector.reduce_sum(out=rhoT[:], in_=tmp3[:].rearrange("p m h -> p h m"), axis=X)
            nc.vector.tensor_scalar(out=phi0[:], in0=rhoT[:], scalar1=float(-C1 / 2.0),
                                    scalar2=float(C0), op0=AL.mult, op1=AL.add)

            # unary = softmax(x)
            nc.sync.dma_start(out=logq[:], in_=xs_d.ap().rearrange("c h w -> h c w"))
            nc.scalar.activation(out=logq[:], in_=logq[:], func=ACTF.Exp)
            nc.vector.tensor_tensor(out=t8[:], in0=logq[:][:, 0:8, :],
                                    in1=logq[:][:, 8:16, :], op=AL.add)
            nc.vector.tensor_tensor(out=t4[:], in0=t8[:][:, 0:4, :],
                                    in1=t8[:][:, 4:8, :], op=AL.add)
            nc.vector.tensor_tensor(out=t2[:], in0=t4[:][:, 0:2, :],
                                    in1=t4[:][:, 2:4, :], op=AL.add)
            nc.vector.tensor_tensor(out=Dsum[:], in0=t2[:][:, 0, :],
                                    in1=t2[:][:, 1, :], op=AL.add)
            nc.vector.reciprocal(out=Rrec[:], in_=Dsum[:])
            nc.vector.tensor_tensor(out=logq[:], in0=logq[:], in1=bc(Rrec[:], C), op=AL.mult)
            nc.vector.tensor_scalar(out=u08m[:], in0=logq[:], scalar1=UNARY_W,
                                    scalar2=UNARY_W, op0=AL.mult, op1=AL.subtract)
            nc.vector.tensor_scalar(out=logq[:], in0=logq[:], scalar1=1.0,
                                    scalar2=1.0, op0=AL.mult, op1=AL.subtract)

        # ---------- steps ----------
        for step in range(NUM_STEPS):
            last = step == NUM_STEPS - 1
            nc.scalar.activation(out=logq[:], in_=logq[:], func=ACTF.Exp)
            nc.vector.tensor_tensor(out=t8[:], in0=logq[:][:, 0:8, :],
                                    in1=logq[:][:, 8:16, :], op=AL.add)
            nc.vector.tensor_tensor(out=t4[:], in0=t8[:][:, 0:4, :],
                                    in1=t8[:][:, 4:8, :], op=AL.add)
            nc.vector.tensor_tensor(out=t2[:], in0=t4[:][:, 0:2, :],
                                    in1=t4[:][:, 2:4, :], op=AL.add)
            nc.vector.tensor_tensor(out=Dsum[:], in0=t2[:][:, 0, :],
                                    in1=t2[:][:, 1, :], op=AL.add)
            nc.vector.reciprocal(out=Rrec[:], in_=Dsum[:])
            nc.vector.tensor_tensor(out=logq[:], in0=logq[:], in1=bc(Rrec[:], C), op=AL.mult)
            qw = sc.tile([SH, C, wb], f32, tag="qw")
            nc.vector.reduce_sum(out=qw[:], in_=logq[:].rearrange("p c (v k) -> p c v k", k=4),
                                 axis=X)
            with tc.tile_pool(name="psv", bufs=1, space="PSUM") as psv:
                vps = psv.tile([SB, C, wb], f32, tag="vps")
                for g in range(4):           # chunk moving free to 512
                    nc.tensor.matmul(vps[:, 4 * g:4 * (g + 1), :], p4s[:],
                                     qw[:, 4 * g:4 * (g + 1), :], start=True, stop=True)
                vcp = sc.tile([SB, C, wb], f32, tag="cpy2")
                nc.vector.tensor_copy(vcp[:], vps[:])
                nc.sync.dma_start(out=vbounce[:], in_=vcp[:])
            nc.gpsimd.collective_compute(
                "AllGather", AL.bypass, replica_groups=[[0, 1, 2, 3], [4, 5, 6, 7]],
                ins=[vbounce[:].opt()], outs=[gpad[:][6:134].opt()])

            # compat (fp32), output directly w-on-partitions:
            # out[w, d] = sum_c V[c, h, w] * w01[c, d] per h row
            nc.sync.dma_start(
                out=vcc[:],
                in_=gpad[:][bass.ds(q32, NH), :, :].rearrange("h c w -> c h w"))
            v0t = sc.tile([wb, C, NH], f32, tag="v0t")
            v1t = sc.tile([wb, C, NH], f32, tag="v1t")
            with tc.tile_pool(name="psc", bufs=1, space="PSUM") as psc:
                cpw = psc.tile([wb, NH, 32], f32, tag="cpw")
                for h in range(NH):
                    nc.tensor.matmul(cpw[:, h, :], vcc[:, h, :], w01[:],
                                     start=True, stop=True)
                nc.vector.tensor_copy(
                    v0t[:], cpw[:][:, :, 0:16].rearrange("w h c -> w c h"))
                nc.vector.tensor_copy(
                    v1t[:], cpw[:][:, :, 16:32].rearrange("w h c -> w c h"))

            flds = []
            for m in range(3):
                f = sc.tile([wb, C, NH], f32, tag=f"fl{m}")
                nc.vector.tensor_tensor(out=f[:], in0=v0t[:], in1=bc(rT[:, m, 1:45], C),
                                        op=AL.mult)
                flds.append(f)
            f4 = sc.tile([wb, C, NH], f32, tag="fl4")
            nc.vector.tensor_tensor(out=f4[:], in0=v0t[:], in1=bc(rhoT[:, 1:45], C),
                                    op=AL.mult)

            msg = sc.tile([wb, C, NO], f32, tag="msg")
            tmpm = sc.tile([wb, 8, NO], f32, tag="tmpm")
            for cf in range(2):          # c-halves: psum + moving free <= 512
              with tc.tile_pool(name="psb", bufs=1, space="PSUM") as psb:
                cs = slice(8 * cf, 8 * (cf + 1))
                specs = (("s0", v0t, gd0), ("s1", flds[0], gd0),
                         ("s2", flds[1], gd0), ("s3", flds[2], gd0),
                         ("s4", f4, gd0), ("sk", v1t, gd1))
                stiles = []
                for nm, _, _ in specs:
                    st = psb.tile([wb, 8, NO], f32, tag=nm)
                    stiles.append(st)
                for k in range(KS):
                    for st, (nm, srct, gdt) in zip(stiles, specs):
                        if gdt is gd1 and not 3 <= k <= 7:
                            continue    # g1 beyond +-2 shifts is <= 7.2e-4
                        ks, ke = (3, 7) if gdt is gd1 else (0, KS - 1)
                        nc.tensor.matmul(st[:], gdt[:, k, :], srct[:, cs, k:k + NO],
                                         start=(k == ks), stop=(k == ke))
                s0, s1, s2, s3, s4, skt = stiles
                mh = msg[:, cs, :]
                nc.vector.tensor_tensor(out=mh, in0=s0[:], in1=bc(phi0[:, 6:6 + NO], 8),
                                        op=AL.mult)
                for m in range(3):
                    nc.vector.tensor_tensor(out=tmpm[:], in0=[s1, s2, s3][m][:],
                                            in1=bc(rT[:, m, 6:6 + NO], 8), op=AL.mult)
                    nc.vector.scalar_tensor_tensor(out=mh, in0=tmpm[:], scalar=float(C1),
                                                   in1=mh, op0=AL.mult, op1=AL.add)
                nc.vector.scalar_tensor_tensor(out=mh, in0=s4[:], scalar=float(-C1 / 2.0),
                                               in1=mh, op0=AL.mult, op1=AL.add)
                nc.vector.tensor_tensor(out=mh, in0=mh, in1=skt[:], op=AL.add)

            tmin = sc.tile([wb, NO], f32, tag="tmin")
            nc.vector.tensor_reduce(out=tmin[:], in_=msg[:].rearrange("p c h -> p h c"),
                                    axis=X, op=AL.min)
            nc.vector.tensor_tensor(out=msgb[:], in0=msg[:], in1=bc(tmin[:], C),
                                    op=AL.subtract)

            for j in range(4):
              with tc.tile_pool(name="psu", bufs=1, space="PSUM") as psu:
                for cf in range(2):
                    pj = psu.tile([wb, 8, NO], f32, tag=f"pj{cf}")
                    nc.tensor.matmul(pj[:],
                                     uwb[:, wb * j: wb * (j + 1)],
                                     msgb[:, 8 * cf:8 * (cf + 1), :], start=True, stop=True)
                    nc.vector.tensor_copy(xwb[:, 8 * cf:8 * (cf + 1), 0:NO], pj[:])
                nc.sync.dma_start(out=xwd[:][j], in_=xwb[:])
                if last:
                    tj = psu.tile([wb, NO], f32, tag="tj")
                    nc.tensor.matmul(tj[:], uw[:, wb * j: wb * (j + 1)], tmin[:],
                                     start=True, stop=True)
                    nc.vector.tensor_copy(tcb[:, 0:NO], tj[:])
                    nc.sync.dma_start(out=twd[:][j], in_=tcb[:])

            nc.sync.dma_start_transpose(
                out=xtb[:].rearrange("h j w c -> h (j w c)"),
                in_=xwd[:].rearrange("j w c h -> (j w c) h"))
            for half in range(2):
                with tc.tile_pool(name="psh", bufs=1, space="PSUM") as psh:
                    xp = psh.tile([SH, 8, W], f32, tag="xp")
                    for cc in range(8):
                        nc.tensor.matmul(
                            xp[:, cc, :], uhl[:],
                            xtb[0:NO, :, :, 8 * half + cc].rearrange(
                                "h j w -> h (j w)"),
                            start=True, stop=True)
                    nc.vector.scalar_tensor_tensor(
                        out=logq[:, 8 * half:8 * (half + 1), :], in0=xp[:], scalar=-1.0,
                        in1=u08m[:, 8 * half:8 * (half + 1), :], op0=AL.mult, op1=AL.add)
            if last:
                tt = sc.tile([wb, W], bf16, tag="tt")
                nc.sync.dma_start_transpose(
                    out=tt[:], in_=twd[:].rearrange("j w h -> (j w) h"))
                with tc.tile_pool(name="pst", bufs=1, space="PSUM") as pst:
                    tp = pst.tile([SH, W], f32, tag="tp")
                    nc.tensor.matmul(tp[:], uhl[:], tt[0:NO, :], start=True, stop=True)
                    upt = sc.tile([SH, W], f32, tag="upt")
                    nc.vector.tensor_scalar(out=upt[:], in0=tp[:], scalar1=-1.0,
                                            scalar2=UNARY_W, op0=AL.mult, op1=AL.add)
                nc.vector.tensor_tensor(out=logq[:], in0=logq[:], in1=bc(upt[:], C),
                                        op=AL.add)

        nc.sync.dma_start(out=out_d.ap().rearrange("c h w -> h c w"), in_=logq[:])

    nc.compile()
    return nc


def kernel(x, image, w_compat0, w_compat1):
    from concourse import bass_utils

    if "nc" not in _CACHE:
        _CACHE["consts"] = _host_consts()
        _CACHE["nc"] = _build()
    nc = _CACHE["nc"]
    cst = _CACHE["consts"]

    x = np.ascontiguousarray(x, np.float32)
    image = np.ascontiguousarray(image, np.float32)
    in_maps = []
    for cid in range(8):
        b, q = cid // 4, cid % 4
        r0 = 128 * q
        ie = np.zeros((3, 184, W), np.float32)
        lo, hi = r0 - 28, r0 + 156
        slo, shi = max(lo, 0), min(hi, H)
        ie[:, slo - lo:shi - lo, :] = image[b, :, slo:shi, :] / np.float32(RGB_SCALE)
        in_maps.append({
            "xs": np.ascontiguousarray(x[b, :, r0:r0 + 128, :]),
            "imge": ie,
            "uh": _bf16(cst["Uh_loc"][q]),
            "uhf": np.ascontiguousarray(cst["Uh_loc"][q]),
            "w0r": np.ascontiguousarray((PW0 * w_compat0).T.astype(np.float32)),
            "w1r": np.ascontiguousarray((PW1 * w_compat1).T.astype(np.float32)),
            "gd0": cst["Gd0"], "gd1": cst["Gd1"], "p4s": cst["P4s"],
            "p4i": cst["P4i"], "uw": cst["Uw"],
        })
    res = bass_utils.run_bass_kernel_spmd(nc, in_maps, core_ids=list(range(8)),
                                          **_CACHE.get("run_kwargs", {}))
    _CACHE["last_result"] = res
    out = np.empty((B, C, H, W), np.float32)
    for cid in range(8):
        b, q = cid // 4, cid % 4
        out[b, :, 128 * q:128 * (q + 1), :] = res.results[cid]["out"]
    return out



# revision 21
# speedup vs baseline: 1.0094x; 1.0094x over previous
"""Trainium2 Bass kernel for nn_CRFModel (PAC-CRF mean-field, 5 steps).

Sharding: 8 cores = batch (2) x h-stripe (4). Full-res softmax/update are
pointwise per stripe; the blur-res pooled softmax V is AllGather'd within
each 4-core batch group every step; the 11x11 pixel-adaptive conv runs as 11
PSUM-accumulated banded matmuls (w-band x h-shift) on a linearized RGB
kernel:  K0 ~= G_spatial * (c0 - c1*||dr||^2/2)  (minimax linear, err<=5e-6).
Kernel 1 is position-only at blur res => exact fixed separable Gaussian.
Bilinear upsample, 4x4 pooling and compat are fp32 PE matmuls.
"""
import numpy as np

def _bf16(x):
    import ml_dtypes
    return np.asarray(x, dtype=np.float32).astype(ml_dtypes.bfloat16)

C = 16; B = 2; H = W = 512; KS = 11; PAD = 5; NUM_STEPS = 5
UNARY_W = 0.8; PW0, PW1 = 2.0, 0.6; RGB_SCALE = 13.0
hb = H // 4; wb = W // 4                 # 128, 128
SH = 128                                 # full-res stripe rows
SB = 32                                  # blur-res stripe rows
NH = 44                                  # blur rows per core (34 out + 10)
NO = 34                                  # blur out rows (32 + 2 bilinear halo)
ZMAX = 3.0 * (1.0 / RGB_SCALE) ** 2 / 2.0
_c1 = (1.0 - np.exp(-ZMAX)) / ZMAX
_zs = -np.log(_c1)
_E = (1.0 - _c1 * _zs - np.exp(-_zs)) / 2.0
C0 = np.float32(1.0 - _E)
C1 = np.float32(_c1)

_CACHE = {}


def _host_consts():
    d = np.arange(-PAD, PAD + 1, dtype=np.float64)
    g0 = np.exp(-(d ** 2) / 800.0)
    g1 = np.exp(-8.0 * (d ** 2) / 9.0)

    def band(g):
        M = np.zeros((wb, wb), np.float32)
        for j in range(wb):
            for k in range(KS):
                i = j + k - PAD
                if 0 <= i < wb:
                    M[i, j] = np.float32(g[k])
        return M

    Gd0 = np.stack([np.float32(g0[k]) * band(g0) for k in range(KS)])
    Gd1 = np.stack([np.float32(g1[k]) * band(g1) for k in range(KS)])

    P4s = np.zeros((SH, SB), np.float32)
    for r in range(SH):
        P4s[r, r // 4] = 1.0 / 16.0

    def up_matrix(n_out, n_in):
        U = np.zeros((n_in, n_out), np.float32)
        s = n_in / n_out
        for r in range(n_out):
            y = (r + 0.5) * s - 0.5
            y0 = int(np.floor(y)); fr = np.float32(y - y0)
            U[min(max(y0, 0), n_in - 1), r] += np.float32(1) - fr
            U[min(max(y0 + 1, 0), n_in - 1), r] += fr
        return U

    Uw = up_matrix(W, wb)
    Uh_full = up_matrix(H, hb)
    Uh_loc = np.zeros((4, NO, SH), np.float32)
    for q in range(4):
        blk = Uh_full[:, SH * q: SH * (q + 1)]
        for i in range(NO):
            k = 32 * q - 1 + i
            if 0 <= k < hb:
                Uh_loc[q, i] = blk[k]
    P4i = np.zeros((92, 23), np.float32)
    for r in range(92):
        P4i[r, r // 4] = 1.0 / 16.0
    return dict(Gd0=Gd0, Gd1=Gd1, P4s=P4s, Uw=np.ascontiguousarray(Uw),
                Uh_loc=Uh_loc, P4i=P4i)


def _build():
    import concourse.bass as bass
    import concourse.bacc as bacc
    import concourse.tile as tile
    from concourse import mybir
    from contextlib import ExitStack

    f32 = mybir.dt.float32
    bf16 = mybir.dt.bfloat16
    AL = mybir.AluOpType
    ACTF = mybir.ActivationFunctionType
    X = mybir.AxisListType.X

    nc = bacc.Bacc("TRN2", target_bir_lowering=False, debug=False, num_devices=8)
    xs_d = nc.dram_tensor("xs", [C, SH, W], f32, kind="ExternalInput")
    img_d = nc.dram_tensor("imge", [3, 184, W], f32, kind="ExternalInput")
    uh_d = nc.dram_tensor("uh", [NO, SH], bf16, kind="ExternalInput")
    uhf_d = nc.dram_tensor("uhf", [NO, SH], f32, kind="ExternalInput")
    w0_d = nc.dram_tensor("w0r", [16, 16], f32, kind="ExternalInput")
    w1_d = nc.dram_tensor("w1r", [16, 16], f32, kind="ExternalInput")
    gd0_d = nc.dram_tensor("gd0", [KS, wb, wb], f32, kind="ExternalInput")
    gd1_d = nc.dram_tensor("gd1", [KS, wb, wb], f32, kind="ExternalInput")
    p4s_d = nc.dram_tensor("p4s", [SH, SB], f32, kind="ExternalInput")
    p4i_d = nc.dram_tensor("p4i", [92, 23], f32, kind="ExternalInput")
    uw_d = nc.dram_tensor("uw", [wb, W], f32, kind="ExternalInput")
    out_d = nc.dram_tensor("out", [C, SH, W], f32, kind="ExternalOutput")

    def bc(ap, n, at=1):
        """insert broadcast dim (step0 x n) at free position `at`."""
        dims = list(ap.ap)
        dims.insert(at, [0, n])
        return bass.AP(tensor=ap.tensor, offset=ap.offset, ap=dims)

    with tile.TileContext(nc) as tc, ExitStack() as ctx:
        sb = ctx.enter_context(tc.tile_pool(name="sb", bufs=1))
        sc = ctx.enter_context(tc.tile_pool(name="sc", bufs=1))
        dr = ctx.enter_context(tc.tile_pool(name="dr", bufs=1, space="DRAM"))

        q32 = nc.sync.partition_id() % 4 * 32

        logq = sb.tile([SH, C, W], f32)
        u08m = sb.tile([SH, C, W], f32)
        gd0 = sb.tile([wb, KS, wb], f32)
        nc.sync.dma_start(out=gd0[:], in_=gd0_d.ap().rearrange("k v w -> v k w"))
        gd1 = sb.tile([wb, KS, wb], f32)
        nc.sync.dma_start(out=gd1[:], in_=gd1_d.ap().rearrange("k v w -> v k w"))
        p4s = sb.tile([SH, SB], f32); nc.sync.dma_start(out=p4s[:], in_=p4s_d.ap())
        uw = sb.tile([wb, W], f32); nc.sync.dma_start(out=uw[:], in_=uw_d.ap())
        uhl = sb.tile([NO, SH], bf16); nc.sync.dma_start(out=uhl[:], in_=uh_d.ap())
        uhlf = sb.tile([NO, SH], f32); nc.sync.dma_start(out=uhlf[:], in_=uhf_d.ap())
        xwb = sb.tile([wb, C, wb], bf16)        # Up_w(msg-tmin) bf16, h padded to 128
        xtb = sb.tile([wb, 4, wb, C], bf16)     # xbar out: [h(34 valid), j, w, c]
        tcb = sb.tile([wb, wb], bf16)           # Up_w(tmin) bf16, h padded
        w01 = sb.tile([16, 32], f32)
        nc.sync.dma_start(out=w01[:, 0:16], in_=w0_d.ap())
        nc.sync.dma_start(out=w01[:, 16:32], in_=w1_d.ap())
        vcc = sb.tile([16, NH, wb], f32)        # gathered V, C-part
        rT = sb.tile([wb, 3, 46], f32)
        rhoT = sb.tile([wb, 46], f32)
        phi0 = sb.tile([wb, 46], f32)
        Dsum = sb.tile([SH, W], f32)
        Rrec = sb.tile([SH, W], f32)
        t8 = sb.tile([SH, 8, W], f32)
        t4 = sb.tile([SH, 4, W], f32)
        t2 = sb.tile([SH, 2, W], f32)

        vbounce = dr.tile([SB, C, wb], f32)
        gpad = dr.tile([140, C, wb], f32)
        v0d = dr.tile([C, NH, wb], f32)
        v1d = dr.tile([C, NH, wb], f32)
        xwd = dr.tile([4, wb, C, wb], bf16)
        twd = dr.tile([4, wb, wb], bf16)

        # ---------- init ----------
        with tc.tile_pool(name="ini", bufs=1) as ini:
            zpad = ini.tile([96, wb], f32)
            nc.vector.memset(zpad[:], 0.0)
            nc.sync.dma_start(out=gpad[:][0:6].rearrange("a b w -> (a b) w"), in_=zpad[:])
            nc.sync.dma_start(out=gpad[:][134:140].rearrange("a b w -> (a b) w"), in_=zpad[:])
            nc.vector.memset(xwb[:], 0.0)
            nc.vector.memset(tcb[:], 0.0)

            p4i = ini.tile([92, 23], f32)
            nc.sync.dma_start(out=p4i[:], in_=p4i_d.ap())
            for ch in range(2):
                imgc = ini.tile([92, 3, W], f32, tag="imgc")
                nc.sync.dma_start(
                    out=imgc[:],
                    in_=img_d.ap()[:, 92 * ch:92 * (ch + 1), :].rearrange("c h w -> h c w"))
                pw_ = ini.tile([92, 3, wb], f32, tag="pw_")
                nc.vector.reduce_sum(
                    out=pw_[:], in_=imgc[:].rearrange("p c (v k) -> p c v k", k=4), axis=X)
                with tc.tile_pool(name="psi", bufs=1, space="PSUM") as psi:
                    ip = psi.tile([23, 3, wb], f32, tag="ip")
                    nc.tensor.matmul(ip[:], p4i[:], pw_[:], start=True, stop=True)
                    ib = dr.tile([23, 3, wb], f32, tag="ib")
                    icp = ini.tile([23, 3, wb], f32, tag="icp")
                    nc.vector.tensor_copy(icp[:], ip[:])
                    nc.sync.dma_start(out=ib[:], in_=icp[:])
                for m3 in range(3):
                    nc.sync.dma_start(out=rT[:, m3, 23 * ch:23 * (ch + 1)],
                                      in_=ib[:][:, m3, :].rearrange("h w -> w h"))
            tmp3 = ini.tile([wb, 3, 46], f32)
            nc.vector.tensor_tensor(out=tmp3[:], in0=rT[:], in1=rT[:], op=AL.mult)
            nc.vector.reduce_sum(out=rhoT[:], in_=tmp3[:].rearrange("p m h -> p h m"), axis=X)
            nc.vector.tensor_scalar(out=phi0[:], in0=rhoT[:], scalar1=float(-C1 / 2.0),
                                    scalar2=float(C0), op0=AL.mult, op1=AL.add)

            # unary = softmax(x)
            nc.sync.dma_start(out=logq[:], in_=xs_d.ap().rearrange("c h w -> h c w"))
            nc.scalar.activation(out=logq[:], in_=logq[:], func=ACTF.Exp)
            nc.vector.tensor_tensor(out=t8[:], in0=logq[:][:, 0:8, :],
                                    in1=logq[:][:, 8:16, :], op=AL.add)
            nc.vector.tensor_tensor(out=t4[:], in0=t8[:][:, 0:4, :],
                                    in1=t8[:][:, 4:8, :], op=AL.add)
            nc.vector.tensor_tensor(out=t2[:], in0=t4[:][:, 0:2, :],
                                    in1=t4[:][:, 2:4, :], op=AL.add)
            nc.vector.tensor_tensor(out=Dsum[:], in0=t2[:][:, 0, :],
                                    in1=t2[:][:, 1, :], op=AL.add)
            nc.vector.reciprocal(out=Rrec[:], in_=Dsum[:])
            nc.vector.tensor_tensor(out=logq[:], in0=logq[:], in1=bc(Rrec[:], C), op=AL.mult)
            nc.vector.tensor_scalar(out=u08m[:], in0=logq[:], scalar1=UNARY_W,
                                    scalar2=UNARY_W, op0=AL.mult, op1=AL.subtract)
            nc.vector.tensor_scalar(out=logq[:], in0=logq[:], scalar1=1.0,
                                    scalar2=1.0, op0=AL.mult, op1=AL.subtract)

        # ---------- steps ----------
        for step in range(NUM_STEPS):
            last = step == NUM_STEPS - 1
            nc.scalar.activation(out=logq[:], in_=logq[:], func=ACTF.Exp)
            nc.vector.tensor_tensor(out=t8[:], in0=logq[:][:, 0:8, :],
                                    in1=logq[:][:, 8:16, :], op=AL.add)
            nc.vector.tensor_tensor(out=t4[:], in0=t8[:][:, 0:4, :],
                                    in1=t8[:][:, 4:8, :], op=AL.add)
            nc.vector.tensor_tensor(out=t2[:], in0=t4[:][:, 0:2, :],
                                    in1=t4[:][:, 2:4, :], op=AL.add)
            nc.vector.tensor_tensor(out=Dsum[:], in0=t2[:][:, 0, :],
                                    in1=t2[:][:, 1, :], op=AL.add)
            nc.vector.reciprocal(out=Rrec[:], in_=Dsum[:])
            nc.vector.tensor_tensor(out=logq[:], in0=logq[:], in1=bc(Rrec[:], C), op=AL.mult)
            qw = sc.tile([SH, C, wb], f32, tag="qw")
            nc.vector.reduce_sum(out=qw[:], in_=logq[:].rearrange("p c (v k) -> p c v k", k=4),
                                 axis=X)
            with tc.tile_pool(name="psv", bufs=1, space="PSUM") as psv:
                vps = psv.tile([SB, C, wb], f32, tag="vps")
                for g in range(4):           # chunk moving free to 512
                    nc.tensor.matmul(vps[:, 4 * g:4 * (g + 1), :], p4s[:],
                                     qw[:, 4 * g:4 * (g + 1), :], start=True, stop=True)
                vcp = sc.tile([SB, C, wb], f32, tag="cpy2")
                nc.vector.tensor_copy(vcp[:], vps[:])
                nc.sync.dma_start(out=vbounce[:], in_=vcp[:])
            nc.gpsimd.collective_compute(
                "AllGather", AL.bypass, replica_groups=[[0, 1, 2, 3], [4, 5, 6, 7]],
                ins=[vbounce[:].opt()], outs=[gpad[:][6:134].opt()])

            # compat (fp32), output directly w-on-partitions:
            # out[w, d] = sum_c V[c, h, w] * w01[c, d] per h row
            nc.sync.dma_start(
                out=vcc[:],
                in_=gpad[:][bass.ds(q32, NH), :, :].rearrange("h c w -> c h w"))
            v0t = sc.tile([wb, C, NH], f32, tag="v0t")
            v1t = sc.tile([wb, C, NH], f32, tag="v1t")
            with tc.tile_pool(name="psc", bufs=1, space="PSUM") as psc:
                cpw = psc.tile([wb, NH, 32], f32, tag="cpw")
                for h in range(NH):
                    nc.tensor.matmul(cpw[:, h, :], vcc[:, h, :], w01[:],
                                     start=True, stop=True)
                nc.vector.tensor_copy(
                    v0t[:], cpw[:][:, :, 0:16].rearrange("w h c -> w c h"))
                nc.vector.tensor_copy(
                    v1t[:], cpw[:][:, :, 16:32].rearrange("w h c -> w c h"))

            flds = []
            for m in range(3):
                f = sc.tile([wb, C, NH], f32, tag=f"fl{m}")
                nc.vector.tensor_tensor(out=f[:], in0=v0t[:], in1=bc(rT[:, m, 1:45], C),
                                        op=AL.mult)
                flds.append(f)
            f4 = sc.tile([wb, C, NH], f32, tag="fl4")
            nc.vector.tensor_tensor(out=f4[:], in0=v0t[:], in1=bc(rhoT[:, 1:45], C),
                                    op=AL.mult)

            msg = sc.tile([wb, C, NO], f32, tag="msg")
            tmpm = sc.tile([wb, 8, NO], f32, tag="tmpm")
            for cf in range(2):          # c-halves: psum + moving free <= 512
              with tc.tile_pool(name="psb", bufs=1, space="PSUM") as psb:
                cs = slice(8 * cf, 8 * (cf + 1))
                stiles = []
                for nm, srct, gdt in (("s0", v0t, gd0), ("s1", flds[0], gd0),
                                     ("s2", flds[1], gd0), ("s3", flds[2], gd0),
                                     ("s4", f4, gd0), ("sk", v1t, gd1)):
                    st = psb.tile([wb, 8, NO], f32, tag=nm)
                    for k in range(KS):
                        nc.tensor.matmul(st[:], gdt[:, k, :], srct[:, cs, k:k + NO],
                                         start=(k == 0), stop=(k == KS - 1))
                    stiles.append(st)
                s0, s1, s2, s3, s4, skt = stiles
                mh = msg[:, cs, :]
                nc.vector.tensor_tensor(out=mh, in0=s0[:], in1=bc(phi0[:, 6:6 + NO], 8),
                                        op=AL.mult)
                for m in range(3):
                    nc.vector.tensor_tensor(out=tmpm[:], in0=[s1, s2, s3][m][:],
                                            in1=bc(rT[:, m, 6:6 + NO], 8), op=AL.mult)
                    nc.vector.scalar_tensor_tensor(out=mh, in0=tmpm[:], scalar=float(C1),
                                                   in1=mh, op0=AL.mult, op1=AL.add)
                nc.vector.scalar_tensor_tensor(out=mh, in0=s4[:], scalar=float(-C1 / 2.0),
                                               in1=mh, op0=AL.mult, op1=AL.add)
                nc.vector.tensor_tensor(out=mh, in0=mh, in1=skt[:], op=AL.add)

            tmin = sc.tile([wb, NO], f32, tag="tmin")
            nc.vector.tensor_reduce(out=tmin[:], in_=msg[:].rearrange("p c h -> p h c"),
                                    axis=X, op=AL.min)
            nc.vector.tensor_tensor(out=msg[:], in0=msg[:], in1=bc(tmin[:], C),
                                    op=AL.subtract)

            for j in range(4):
              with tc.tile_pool(name="psu", bufs=1, space="PSUM") as psu:
                for cf in range(2):
                    pj = psu.tile([wb, 8, NO], f32, tag=f"pj{cf}")
                    nc.tensor.matmul(pj[:],
                                     uw[:, wb * j: wb * (j + 1)],
                                     msg[:, 8 * cf:8 * (cf + 1), :], start=True, stop=True)
                    nc.vector.tensor_copy(xwb[:, 8 * cf:8 * (cf + 1), 0:NO], pj[:])
                nc.sync.dma_start(out=xwd[:][j], in_=xwb[:])
                if last:
                    tj = psu.tile([wb, NO], f32, tag="tj")
                    nc.tensor.matmul(tj[:], uw[:, wb * j: wb * (j + 1)], tmin[:],
                                     start=True, stop=True)
                    nc.vector.tensor_copy(tcb[:, 0:NO], tj[:])
                    nc.sync.dma_start(out=twd[:][j], in_=tcb[:])

            nc.sync.dma_start_transpose(
                out=xtb[:].rearrange("h j w c -> h (j w c)"),
                in_=xwd[:].rearrange("j w c h -> (j w c) h"))
            for half in range(2):
                with tc.tile_pool(name="psh", bufs=1, space="PSUM") as psh:
                    xp = psh.tile([SH, 8, W], f32, tag="xp")
                    for cc in range(8):
                        nc.tensor.matmul(
                            xp[:, cc, :], uhl[:],
                            xtb[0:NO, :, :, 8 * half + cc].rearrange(
                                "h j w -> h (j w)"),
                            start=True, stop=True)
                    nc.vector.scalar_tensor_tensor(
                        out=logq[:, 8 * half:8 * (half + 1), :], in0=xp[:], scalar=-1.0,
                        in1=u08m[:, 8 * half:8 * (half + 1), :], op0=AL.mult, op1=AL.add)
            if last:
                tt = sc.tile([wb, W], bf16, tag="tt")
                nc.sync.dma_start_transpose(
                    out=tt[:], in_=twd[:].rearrange("j w h -> (j w) h"))
                with tc.tile_pool(name="pst", bufs=1, space="PSUM") as pst:
                    tp = pst.tile([SH, W], f32, tag="tp")
                    nc.tensor.matmul(tp[:], uhl[:], tt[0:NO, :], start=True, stop=True)
                    upt = sc.tile([SH, W], f32, tag="upt")
                    nc.vector.tensor_scalar(out=upt[:], in0=tp[:], scalar1=-1.0,
                                            scalar2=UNARY_W, op0=AL.mult, op1=AL.add)
                nc.vector.tensor_tensor(out=logq[:], in0=logq[:], in1=bc(upt[:], C),
                                        op=AL.add)

        nc.sync.dma_start(out=out_d.ap().rearrange("c h w -> h c w"), in_=logq[:])

    nc.compile()
    return nc


def kernel(x, image, w_compat0, w_compat1):
    from concourse import bass_utils

    if "nc" not in _CACHE:
        _CACHE["consts"] = _host_consts()
        _CACHE["nc"] = _build()
    nc = _CACHE["nc"]
    cst = _CACHE["consts"]

    x = np.ascontiguousarray(x, np.float32)
    image = np.ascontiguousarray(image, np.float32)
    in_maps = []
    for cid in range(8):
        b, q = cid // 4, cid % 4
        r0 = 128 * q
        ie = np.zeros((3, 184, W), np.float32)
        lo, hi = r0 - 28, r0 + 156
        slo, shi = max(lo, 0), min(hi, H)
        ie[:, slo - lo:shi - lo, :] = image[b, :, slo:shi, :] / np.float32(RGB_SCALE)
        in_maps.append({
            "xs": np.ascontiguousarray(x[b, :, r0:r0 + 128, :]),
            "imge": ie,
            "uh": _bf16(cst["Uh_loc"][q]),
            "uhf": np.ascontiguousarray(cst["Uh_loc"][q]),
            "w0r": np.ascontiguousarray((PW0 * w_compat0).T.astype(np.float32)),
            "w1r": np.ascontiguousarray((PW1 * w_compat1).T.astype(np.float32)),
            "gd0": cst["Gd0"], "gd1": cst["Gd1"], "p4s": cst["P4s"],
            "p4i": cst["P4i"], "uw": cst["Uw"],
        })
    res = bass_utils.run_bass_kernel_spmd(nc, in_maps, core_ids=list(range(8)),
                                          **_CACHE.get("run_kwargs", {}))
    _CACHE["last_result"] = res
    out = np.empty((B, C, H, W), np.float32)
    for cid in range(8):
        b, q = cid // 4, cid % 4
        out[b, :, 128 * q:128 * (q + 1), :] = res.results[cid]["out"]
    return out



# revision 22
# speedup vs baseline: 1.0736x; 1.0636x over previous
"""Trainium2 Bass kernel for nn_CRFModel (PAC-CRF mean-field, 5 steps).

Sharding: 8 cores = batch (2) x h-stripe (4). Full-res softmax/update are
pointwise per stripe; the blur-res pooled softmax V is AllGather'd within
each 4-core batch group every step; the 11x11 pixel-adaptive conv runs as 11
PSUM-accumulated banded matmuls (w-band x h-shift) on a linearized RGB
kernel:  K0 ~= G_spatial * (c0 - c1*||dr||^2/2)  (minimax linear, err<=5e-6).
Kernel 1 is position-only at blur res => exact fixed separable Gaussian.
Bilinear upsample, 4x4 pooling and compat are fp32 PE matmuls.
"""
import numpy as np

def _bf16(x):
    import ml_dtypes
    return np.asarray(x, dtype=np.float32).astype(ml_dtypes.bfloat16)

C = 16; B = 2; H = W = 512; KS = 11; PAD = 5; NUM_STEPS = 5
UNARY_W = 0.8; PW0, PW1 = 2.0, 0.6; RGB_SCALE = 13.0
hb = H // 4; wb = W // 4                 # 128, 128
SH = 128                                 # full-res stripe rows
SB = 32                                  # blur-res stripe rows
NH = 44                                  # blur rows per core (34 out + 10)
NO = 34                                  # blur out rows (32 + 2 bilinear halo)
ZMAX = 3.0 * (1.0 / RGB_SCALE) ** 2 / 2.0
_c1 = (1.0 - np.exp(-ZMAX)) / ZMAX
_zs = -np.log(_c1)
_E = (1.0 - _c1 * _zs - np.exp(-_zs)) / 2.0
C0 = np.float32(1.0 - _E)
C1 = np.float32(_c1)

_CACHE = {}


def _host_consts():
    d = np.arange(-PAD, PAD + 1, dtype=np.float64)
    g0 = np.exp(-(d ** 2) / 800.0)
    g1 = np.exp(-8.0 * (d ** 2) / 9.0)

    def band(g):
        M = np.zeros((wb, wb), np.float32)
        for j in range(wb):
            for k in range(KS):
                i = j + k - PAD
                if 0 <= i < wb:
                    M[i, j] = np.float32(g[k])
        return M

    Gd0 = np.stack([np.float32(g0[k]) * band(g0) for k in range(KS)])
    Gd1 = np.stack([np.float32(g1[k]) * band(g1) for k in range(KS)])

    P4s = np.zeros((SH, SB), np.float32)
    for r in range(SH):
        P4s[r, r // 4] = 1.0 / 16.0

    def up_matrix(n_out, n_in):
        U = np.zeros((n_in, n_out), np.float32)
        s = n_in / n_out
        for r in range(n_out):
            y = (r + 0.5) * s - 0.5
            y0 = int(np.floor(y)); fr = np.float32(y - y0)
            U[min(max(y0, 0), n_in - 1), r] += np.float32(1) - fr
            U[min(max(y0 + 1, 0), n_in - 1), r] += fr
        return U

    Uw = up_matrix(W, wb)
    Uh_full = up_matrix(H, hb)
    Uh_loc = np.zeros((4, NO, SH), np.float32)
    for q in range(4):
        blk = Uh_full[:, SH * q: SH * (q + 1)]
        for i in range(NO):
            k = 32 * q - 1 + i
            if 0 <= k < hb:
                Uh_loc[q, i] = blk[k]
    P4i = np.zeros((92, 23), np.float32)
    for r in range(92):
        P4i[r, r // 4] = 1.0 / 16.0
    return dict(Gd0=Gd0, Gd1=Gd1, P4s=P4s, Uw=np.ascontiguousarray(Uw),
                Uh_loc=Uh_loc, P4i=P4i)


def _build():
    import concourse.bass as bass
    import concourse.bacc as bacc
    import concourse.tile as tile
    from concourse import mybir
    from contextlib import ExitStack

    f32 = mybir.dt.float32
    bf16 = mybir.dt.bfloat16
    AL = mybir.AluOpType
    ACTF = mybir.ActivationFunctionType
    X = mybir.AxisListType.X

    nc = bacc.Bacc("TRN2", target_bir_lowering=False, debug=False, num_devices=8)
    xs_d = nc.dram_tensor("xs", [C, SH, W], f32, kind="ExternalInput")
    img_d = nc.dram_tensor("imge", [3, 184, W], f32, kind="ExternalInput")
    uh_d = nc.dram_tensor("uh", [NO, SH], bf16, kind="ExternalInput")
    uhf_d = nc.dram_tensor("uhf", [NO, SH], f32, kind="ExternalInput")
    w0_d = nc.dram_tensor("w0r", [16, 16], f32, kind="ExternalInput")
    w1_d = nc.dram_tensor("w1r", [16, 16], f32, kind="ExternalInput")
    gd0_d = nc.dram_tensor("gd0", [KS, wb, wb], f32, kind="ExternalInput")
    gd1_d = nc.dram_tensor("gd1", [KS, wb, wb], f32, kind="ExternalInput")
    p4s_d = nc.dram_tensor("p4s", [SH, SB], f32, kind="ExternalInput")
    p4i_d = nc.dram_tensor("p4i", [92, 23], f32, kind="ExternalInput")
    uw_d = nc.dram_tensor("uw", [wb, W], f32, kind="ExternalInput")
    out_d = nc.dram_tensor("out", [C, SH, W], f32, kind="ExternalOutput")

    def bc(ap, n, at=1):
        """insert broadcast dim (step0 x n) at free position `at`."""
        dims = list(ap.ap)
        dims.insert(at, [0, n])
        return bass.AP(tensor=ap.tensor, offset=ap.offset, ap=dims)

    with tile.TileContext(nc) as tc, ExitStack() as ctx:
        sb = ctx.enter_context(tc.tile_pool(name="sb", bufs=1))
        sc = ctx.enter_context(tc.tile_pool(name="sc", bufs=1))
        dr = ctx.enter_context(tc.tile_pool(name="dr", bufs=1, space="DRAM"))

        q32 = nc.sync.partition_id() % 4 * 32

        logq = sb.tile([SH, C, W], f32)
        u08m = sb.tile([SH, C, W], f32)
        gd0 = sb.tile([wb, KS, wb], f32)
        nc.sync.dma_start(out=gd0[:], in_=gd0_d.ap().rearrange("k v w -> v k w"))
        gd1 = sb.tile([wb, KS, wb], f32)
        nc.sync.dma_start(out=gd1[:], in_=gd1_d.ap().rearrange("k v w -> v k w"))
        p4s = sb.tile([SH, SB], f32); nc.sync.dma_start(out=p4s[:], in_=p4s_d.ap())
        uw = sb.tile([wb, W], f32); nc.sync.dma_start(out=uw[:], in_=uw_d.ap())
        uhl = sb.tile([NO, SH], bf16); nc.sync.dma_start(out=uhl[:], in_=uh_d.ap())
        uhlf = sb.tile([NO, SH], f32); nc.sync.dma_start(out=uhlf[:], in_=uhf_d.ap())
        xwb = sb.tile([wb, C, wb], bf16)        # Up_w(msg-tmin) bf16, h padded to 128
        xtb = sb.tile([wb, 4, wb, C], bf16)     # xbar out: [h(34 valid), j, w, c]
        tcb = sb.tile([wb, wb], bf16)           # Up_w(tmin) bf16, h padded
        w01 = sb.tile([16, 32], f32)
        nc.sync.dma_start(out=w01[:, 0:16], in_=w0_d.ap())
        nc.sync.dma_start(out=w01[:, 16:32], in_=w1_d.ap())
        vcc = sb.tile([16, NH, wb], f32)        # gathered V, C-part
        rT = sb.tile([wb, 3, 46], f32)
        rhoT = sb.tile([wb, 46], f32)
        phi0 = sb.tile([wb, 46], f32)
        Dsum = sb.tile([SH, W], f32)
        Rrec = sb.tile([SH, W], f32)
        t8 = sb.tile([SH, 8, W], f32)
        t4 = sb.tile([SH, 4, W], f32)
        t2 = sb.tile([SH, 2, W], f32)

        vbounce = dr.tile([SB, C, wb], f32)
        gpad = dr.tile([140, C, wb], f32)
        v0d = dr.tile([C, NH, wb], f32)
        v1d = dr.tile([C, NH, wb], f32)
        xwd = dr.tile([4, wb, C, wb], bf16)
        twd = dr.tile([4, wb, wb], bf16)

        # ---------- init ----------
        with tc.tile_pool(name="ini", bufs=1) as ini:
            zpad = ini.tile([96, wb], f32)
            nc.vector.memset(zpad[:], 0.0)
            nc.sync.dma_start(out=gpad[:][0:6].rearrange("a b w -> (a b) w"), in_=zpad[:])
            nc.sync.dma_start(out=gpad[:][134:140].rearrange("a b w -> (a b) w"), in_=zpad[:])
            nc.vector.memset(xwb[:], 0.0)
            nc.vector.memset(tcb[:], 0.0)

            p4i = ini.tile([92, 23], f32)
            nc.sync.dma_start(out=p4i[:], in_=p4i_d.ap())
            for ch in range(2):
                imgc = ini.tile([92, 3, W], f32, tag="imgc")
                nc.sync.dma_start(
                    out=imgc[:],
                    in_=img_d.ap()[:, 92 * ch:92 * (ch + 1), :].rearrange("c h w -> h c w"))
                pw_ = ini.tile([92, 3, wb], f32, tag="pw_")
                nc.vector.reduce_sum(
                    out=pw_[:], in_=imgc[:].rearrange("p c (v k) -> p c v k", k=4), axis=X)
                with tc.tile_pool(name="psi", bufs=1, space="PSUM") as psi:
                    ip = psi.tile([23, 3, wb], f32, tag="ip")
                    nc.tensor.matmul(ip[:], p4i[:], pw_[:], start=True, stop=True)
                    ib = dr.tile([23, 3, wb], f32, tag="ib")
                    icp = ini.tile([23, 3, wb], f32, tag="icp")
                    nc.vector.tensor_copy(icp[:], ip[:])
                    nc.sync.dma_start(out=ib[:], in_=icp[:])
                for m3 in range(3):
                    nc.sync.dma_start(out=rT[:, m3, 23 * ch:23 * (ch + 1)],
                                      in_=ib[:][:, m3, :].rearrange("h w -> w h"))
            tmp3 = ini.tile([wb, 3, 46], f32)
            nc.vector.tensor_tensor(out=tmp3[:], in0=rT[:], in1=rT[:], op=AL.mult)
            nc.vector.reduce_sum(out=rhoT[:], in_=tmp3[:].rearrange("p m h -> p h m"), axis=X)
            nc.vector.tensor_scalar(out=phi0[:], in0=rhoT[:], scalar1=float(-C1 / 2.0),
                                    scalar2=float(C0), op0=AL.mult, op1=AL.add)

            # unary = softmax(x)
            nc.sync.dma_start(out=logq[:], in_=xs_d.ap().rearrange("c h w -> h c w"))
            nc.scalar.activation(out=logq[:], in_=logq[:], func=ACTF.Exp)
            nc.vector.tensor_tensor(out=t8[:], in0=logq[:][:, 0:8, :],
                                    in1=logq[:][:, 8:16, :], op=AL.add)
            nc.vector.tensor_tensor(out=t4[:], in0=t8[:][:, 0:4, :],
                                    in1=t8[:][:, 4:8, :], op=AL.add)
            nc.vector.tensor_tensor(out=t2[:], in0=t4[:][:, 0:2, :],
                                    in1=t4[:][:, 2:4, :], op=AL.add)
            nc.vector.tensor_tensor(out=Dsum[:], in0=t2[:][:, 0, :],
                                    in1=t2[:][:, 1, :], op=AL.add)
            nc.vector.reciprocal(out=Rrec[:], in_=Dsum[:])
            nc.vector.tensor_tensor(out=logq[:], in0=logq[:], in1=bc(Rrec[:], C), op=AL.mult)
            nc.vector.tensor_scalar(out=u08m[:], in0=logq[:], scalar1=UNARY_W,
                                    scalar2=UNARY_W, op0=AL.mult, op1=AL.subtract)
            nc.vector.tensor_scalar(out=logq[:], in0=logq[:], scalar1=1.0,
                                    scalar2=1.0, op0=AL.mult, op1=AL.subtract)

        # ---------- steps ----------
        for step in range(NUM_STEPS):
            last = step == NUM_STEPS - 1
            nc.scalar.activation(out=logq[:], in_=logq[:], func=ACTF.Exp)
            nc.vector.tensor_tensor(out=t8[:], in0=logq[:][:, 0:8, :],
                                    in1=logq[:][:, 8:16, :], op=AL.add)
            nc.vector.tensor_tensor(out=t4[:], in0=t8[:][:, 0:4, :],
                                    in1=t8[:][:, 4:8, :], op=AL.add)
            nc.vector.tensor_tensor(out=t2[:], in0=t4[:][:, 0:2, :],
                                    in1=t4[:][:, 2:4, :], op=AL.add)
            nc.vector.tensor_tensor(out=Dsum[:], in0=t2[:][:, 0, :],
                                    in1=t2[:][:, 1, :], op=AL.add)
            nc.vector.reciprocal(out=Rrec[:], in_=Dsum[:])
            nc.vector.tensor_tensor(out=logq[:], in0=logq[:], in1=bc(Rrec[:], C), op=AL.mult)
            qw = sc.tile([SH, C, wb], f32, tag="qw")
            nc.vector.reduce_sum(out=qw[:], in_=logq[:].rearrange("p c (v k) -> p c v k", k=4),
                                 axis=X)
            with tc.tile_pool(name="psv", bufs=1, space="PSUM") as psv:
                vps = psv.tile([SB, C, wb], f32, tag="vps")
                for g in range(4):           # chunk moving free to 512
                    nc.tensor.matmul(vps[:, 4 * g:4 * (g + 1), :], p4s[:],
                                     qw[:, 4 * g:4 * (g + 1), :], start=True, stop=True)
                vcp = sc.tile([SB, C, wb], f32, tag="cpy2")
                nc.vector.tensor_copy(vcp[:], vps[:])
                nc.sync.dma_start(out=vbounce[:], in_=vcp[:])
            nc.gpsimd.collective_compute(
                "AllGather", AL.bypass, replica_groups=[[0, 1, 2, 3], [4, 5, 6, 7]],
                ins=[vbounce[:].opt()], outs=[gpad[:][6:134].opt()])

            # compat (fp32), output directly w-on-partitions:
            # out[w, d] = sum_c V[c, h, w] * w01[c, d] per h row
            nc.scalar.dma_start(
                out=vcc[:, 6:38, :],
                in_=vbounce[:].rearrange("h c w -> c h w"))
            nc.sync.dma_start(
                out=vcc[:, 0:6, :],
                in_=gpad[:][bass.ds(q32, 6), :, :].rearrange("h c w -> c h w"))
            nc.sync.dma_start(
                out=vcc[:, 38:44, :],
                in_=gpad[:][bass.ds(q32 + 38, 6), :, :].rearrange("h c w -> c h w"))
            v0t = sc.tile([wb, C, NH], f32, tag="v0t")
            v1t = sc.tile([wb, C, NH], f32, tag="v1t")
            with tc.tile_pool(name="psc", bufs=1, space="PSUM") as psc:
                cpw = psc.tile([wb, NH, 32], f32, tag="cpw")
                for h in list(range(6, 38)) + list(range(0, 6)) + list(range(38, NH)):
                    nc.tensor.matmul(cpw[:, h, :], vcc[:, h, :], w01[:],
                                     start=True, stop=True)
                nc.vector.tensor_copy(
                    v0t[:], cpw[:][:, :, 0:16].rearrange("w h c -> w c h"))
                nc.vector.tensor_copy(
                    v1t[:], cpw[:][:, :, 16:32].rearrange("w h c -> w c h"))

            flds = []
            for m in range(3):
                f = sc.tile([wb, C, NH], f32, tag=f"fl{m}")
                nc.vector.tensor_tensor(out=f[:], in0=v0t[:], in1=bc(rT[:, m, 1:45], C),
                                        op=AL.mult)
                flds.append(f)
            f4 = sc.tile([wb, C, NH], f32, tag="fl4")
            nc.vector.tensor_tensor(out=f4[:], in0=v0t[:], in1=bc(rhoT[:, 1:45], C),
                                    op=AL.mult)

            msg = sc.tile([wb, C, NO], f32, tag="msg")
            tmpm = sc.tile([wb, 8, NO], f32, tag="tmpm")
            for cf in range(2):          # c-halves: psum + moving free <= 512
              with tc.tile_pool(name="psb", bufs=1, space="PSUM") as psb:
                cs = slice(8 * cf, 8 * (cf + 1))
                stiles = []
                for nm, srct, gdt in (("s0", v0t, gd0), ("s1", flds[0], gd0),
                                     ("s2", flds[1], gd0), ("s3", flds[2], gd0),
                                     ("s4", f4, gd0), ("sk", v1t, gd1)):
                    st = psb.tile([wb, 8, NO], f32, tag=nm)
                    for k in range(KS):
                        nc.tensor.matmul(st[:], gdt[:, k, :], srct[:, cs, k:k + NO],
                                         start=(k == 0), stop=(k == KS - 1))
                    stiles.append(st)
                s0, s1, s2, s3, s4, skt = stiles
                mh = msg[:, cs, :]
                nc.vector.tensor_tensor(out=mh, in0=s0[:], in1=bc(phi0[:, 6:6 + NO], 8),
                                        op=AL.mult)
                for m in range(3):
                    nc.vector.tensor_tensor(out=tmpm[:], in0=[s1, s2, s3][m][:],
                                            in1=bc(rT[:, m, 6:6 + NO], 8), op=AL.mult)
                    nc.vector.scalar_tensor_tensor(out=mh, in0=tmpm[:], scalar=float(C1),
                                                   in1=mh, op0=AL.mult, op1=AL.add)
                nc.vector.scalar_tensor_tensor(out=mh, in0=s4[:], scalar=float(-C1 / 2.0),
                                               in1=mh, op0=AL.mult, op1=AL.add)
                nc.vector.tensor_tensor(out=mh, in0=mh, in1=skt[:], op=AL.add)

            tmin = sc.tile([wb, NO], f32, tag="tmin")
            nc.vector.tensor_reduce(out=tmin[:], in_=msg[:].rearrange("p c h -> p h c"),
                                    axis=X, op=AL.min)
            nc.vector.tensor_tensor(out=msg[:], in0=msg[:], in1=bc(tmin[:], C),
                                    op=AL.subtract)

            for j in range(4):
              with tc.tile_pool(name="psu", bufs=1, space="PSUM") as psu:
                for cf in range(2):
                    pj = psu.tile([wb, 8, NO], f32, tag=f"pj{cf}")
                    nc.tensor.matmul(pj[:],
                                     uw[:, wb * j: wb * (j + 1)],
                                     msg[:, 8 * cf:8 * (cf + 1), :], start=True, stop=True)
                    nc.vector.tensor_copy(xwb[:, 8 * cf:8 * (cf + 1), 0:NO], pj[:])
                nc.sync.dma_start(out=xwd[:][j], in_=xwb[:])
                if last:
                    tj = psu.tile([wb, NO], f32, tag="tj")
                    nc.tensor.matmul(tj[:], uw[:, wb * j: wb * (j + 1)], tmin[:],
                                     start=True, stop=True)
                    nc.vector.tensor_copy(tcb[:, 0:NO], tj[:])
                    nc.sync.dma_start(out=twd[:][j], in_=tcb[:])

            nc.sync.dma_start_transpose(
                out=xtb[:][:, 0:2, :, :].rearrange("h j w c -> h (j w c)"),
                in_=xwd[:][0:2].rearrange("j w c h -> (j w c) h"))
            nc.scalar.dma_start_transpose(
                out=xtb[:][:, 2:4, :, :].rearrange("h j w c -> h (j w c)"),
                in_=xwd[:][2:4].rearrange("j w c h -> (j w c) h"))
            for half in range(2):
                with tc.tile_pool(name="psh", bufs=1, space="PSUM") as psh:
                    xp = psh.tile([SH, 8, W], f32, tag="xp")
                    for cc in range(8):
                        nc.tensor.matmul(
                            xp[:, cc, :], uhl[:],
                            xtb[0:NO, :, :, 8 * half + cc].rearrange(
                                "h j w -> h (j w)"),
                            start=True, stop=True)
                    nc.vector.scalar_tensor_tensor(
                        out=logq[:, 8 * half:8 * (half + 1), :], in0=xp[:], scalar=-1.0,
                        in1=u08m[:, 8 * half:8 * (half + 1), :], op0=AL.mult, op1=AL.add)
            if last:
                tt = sc.tile([wb, W], bf16, tag="tt")
                nc.sync.dma_start_transpose(
                    out=tt[:], in_=twd[:].rearrange("j w h -> (j w) h"))
                with tc.tile_pool(name="pst", bufs=1, space="PSUM") as pst:
                    tp = pst.tile([SH, W], f32, tag="tp")
                    nc.tensor.matmul(tp[:], uhl[:], tt[0:NO, :], start=True, stop=True)
                    upt = sc.tile([SH, W], f32, tag="upt")
                    nc.vector.tensor_scalar(out=upt[:], in0=tp[:], scalar1=-1.0,
                                            scalar2=UNARY_W, op0=AL.mult, op1=AL.add)
                nc.vector.tensor_tensor(out=logq[:], in0=logq[:], in1=bc(upt[:], C),
                                        op=AL.add)

        nc.sync.dma_start(out=out_d.ap().rearrange("c h w -> h c w"), in_=logq[:])

    nc.compile()
    return nc


def kernel(x, image, w_compat0, w_compat1):
    from concourse import bass_utils

    if "nc" not in _CACHE:
        _CACHE["consts"] = _host_consts()
        _CACHE["nc"] = _build()
    nc = _CACHE["nc"]
    cst = _CACHE["consts"]

    x = np.ascontiguousarray(x, np.float32)
    image = np.ascontiguousarray(image, np.float32)
    in_maps = []
    for cid in range(8):
        b, q = cid // 4, cid % 4
        r0 = 128 * q
        ie = np.zeros((3, 184, W), np.float32)
        lo, hi = r0 - 28, r0 + 156
        slo, shi = max(lo, 0), min(hi, H)
        ie[:, slo - lo:shi - lo, :] = image[b, :, slo:shi, :] / np.float32(RGB_SCALE)
        in_maps.append({
            "xs": np.ascontiguousarray(x[b, :, r0:r0 + 128, :]),
            "imge": ie,
            "uh": _bf16(cst["Uh_loc"][q]),
            "uhf": np.ascontiguousarray(cst["Uh_loc"][q]),
            "w0r": np.ascontiguousarray((PW0 * w_compat0).T.astype(np.float32)),
            "w1r": np.ascontiguousarray((PW1 * w_compat1).T.astype(np.float32)),
            "gd0": cst["Gd0"], "gd1": cst["Gd1"], "p4s": cst["P4s"],
            "p4i": cst["P4i"], "uw": cst["Uw"],
        })
    res = bass_utils.run_bass_kernel_spmd(nc, in_maps, core_ids=list(range(8)),
                                          **_CACHE.get("run_kwargs", {}))
    _CACHE["last_result"] = res
    out = np.empty((B, C, H, W), np.float32)
    for cid in range(8):
        b, q = cid // 4, cid % 4
        out[b, :, 128 * q:128 * (q + 1), :] = res.results[cid]["out"]
    return out



# revision 23
# speedup vs baseline: 1.1148x; 1.0384x over previous
"""Trainium2 Bass kernel for nn_CRFModel (PAC-CRF mean-field, 5 steps).

Sharding: 8 cores = batch (2) x h-stripe (4). Full-res softmax/update are
pointwise per stripe; the blur-res pooled softmax V is AllGather'd within
each 4-core batch group every step; the 11x11 pixel-adaptive conv runs as 11
PSUM-accumulated banded matmuls (w-band x h-shift) on a linearized RGB
kernel:  K0 ~= G_spatial * (c0 - c1*||dr||^2/2)  (minimax linear, err<=5e-6).
Kernel 1 is position-only at blur res => exact fixed separable Gaussian.
Bilinear upsample, 4x4 pooling and compat are fp32 PE matmuls.
"""
import numpy as np

def _bf16(x):
    import ml_dtypes
    return np.asarray(x, dtype=np.float32).astype(ml_dtypes.bfloat16)

C = 16; B = 2; H = W = 512; KS = 11; PAD = 5; NUM_STEPS = 5
UNARY_W = 0.8; PW0, PW1 = 2.0, 0.6; RGB_SCALE = 13.0
hb = H // 4; wb = W // 4                 # 128, 128
SH = 128                                 # full-res stripe rows
SB = 32                                  # blur-res stripe rows
NH = 44                                  # blur rows per core (34 out + 10)
NO = 34                                  # blur out rows (32 + 2 bilinear halo)
ZMAX = 3.0 * (1.0 / RGB_SCALE) ** 2 / 2.0
_c1 = (1.0 - np.exp(-ZMAX)) / ZMAX
_zs = -np.log(_c1)
_E = (1.0 - _c1 * _zs - np.exp(-_zs)) / 2.0
C0 = np.float32(1.0 - _E)
C1 = np.float32(_c1)

_CACHE = {}


def _host_consts():
    d = np.arange(-PAD, PAD + 1, dtype=np.float64)
    g0 = np.exp(-(d ** 2) / 800.0)
    g1 = np.exp(-8.0 * (d ** 2) / 9.0)

    def band(g):
        M = np.zeros((wb, wb), np.float32)
        for j in range(wb):
            for k in range(KS):
                i = j + k - PAD
                if 0 <= i < wb:
                    M[i, j] = np.float32(g[k])
        return M

    Gd0 = np.stack([np.float32(g0[k]) * band(g0) for k in range(KS)])
    Gd1 = np.stack([np.float32(g1[k]) * band(g1) for k in range(KS)])

    P4s = np.zeros((SH, SB), np.float32)
    for r in range(SH):
        P4s[r, r // 4] = 1.0 / 16.0

    def up_matrix(n_out, n_in):
        U = np.zeros((n_in, n_out), np.float32)
        s = n_in / n_out
        for r in range(n_out):
            y = (r + 0.5) * s - 0.5
            y0 = int(np.floor(y)); fr = np.float32(y - y0)
            U[min(max(y0, 0), n_in - 1), r] += np.float32(1) - fr
            U[min(max(y0 + 1, 0), n_in - 1), r] += fr
        return U

    Uw = up_matrix(W, wb)
    Uh_full = up_matrix(H, hb)
    Uh_loc = np.zeros((4, NO, SH), np.float32)
    for q in range(4):
        blk = Uh_full[:, SH * q: SH * (q + 1)]
        for i in range(NO):
            k = 32 * q - 1 + i
            if 0 <= k < hb:
                Uh_loc[q, i] = blk[k]
    P4i = np.zeros((92, 23), np.float32)
    for r in range(92):
        P4i[r, r // 4] = 1.0 / 16.0
    return dict(Gd0=Gd0, Gd1=Gd1, P4s=P4s, Uw=np.ascontiguousarray(Uw),
                Uh_loc=Uh_loc, P4i=P4i)


def _build():
    import concourse.bass as bass
    import concourse.bacc as bacc
    import concourse.tile as tile
    from concourse import mybir
    from contextlib import ExitStack

    f32 = mybir.dt.float32
    bf16 = mybir.dt.bfloat16
    AL = mybir.AluOpType
    ACTF = mybir.ActivationFunctionType
    X = mybir.AxisListType.X

    nc = bacc.Bacc("TRN2", target_bir_lowering=False, debug=False, num_devices=8)
    xs_d = nc.dram_tensor("xs", [C, SH, W], f32, kind="ExternalInput")
    img_d = nc.dram_tensor("imge", [3, 184, W], f32, kind="ExternalInput")
    uh_d = nc.dram_tensor("uh", [NO, SH], bf16, kind="ExternalInput")
    uhf_d = nc.dram_tensor("uhf", [NO, SH], f32, kind="ExternalInput")
    w0_d = nc.dram_tensor("w0r", [16, 16], f32, kind="ExternalInput")
    w1_d = nc.dram_tensor("w1r", [16, 16], f32, kind="ExternalInput")
    gd0_d = nc.dram_tensor("gd0", [KS, wb, wb], f32, kind="ExternalInput")
    gd1_d = nc.dram_tensor("gd1", [KS, wb, wb], f32, kind="ExternalInput")
    p4s_d = nc.dram_tensor("p4s", [SH, SB], f32, kind="ExternalInput")
    p4i_d = nc.dram_tensor("p4i", [92, 23], f32, kind="ExternalInput")
    uw_d = nc.dram_tensor("uw", [wb, W], f32, kind="ExternalInput")
    out_d = nc.dram_tensor("out", [C, SH, W], f32, kind="ExternalOutput")

    def bc(ap, n, at=1):
        """insert broadcast dim (step0 x n) at free position `at`."""
        dims = list(ap.ap)
        dims.insert(at, [0, n])
        return bass.AP(tensor=ap.tensor, offset=ap.offset, ap=dims)

    with tile.TileContext(nc) as tc, ExitStack() as ctx:
        sb = ctx.enter_context(tc.tile_pool(name="sb", bufs=1))
        sc = ctx.enter_context(tc.tile_pool(name="sc", bufs=1))
        dr = ctx.enter_context(tc.tile_pool(name="dr", bufs=1, space="DRAM"))

        q32 = nc.sync.partition_id() % 4 * 32

        logq = sb.tile([SH, C, W], f32)
        u08m = sb.tile([SH, C, W], f32)
        gd0 = sb.tile([wb, KS, wb], f32)
        nc.sync.dma_start(out=gd0[:], in_=gd0_d.ap().rearrange("k v w -> v k w"))
        gd1 = sb.tile([wb, KS, wb], f32)
        nc.sync.dma_start(out=gd1[:], in_=gd1_d.ap().rearrange("k v w -> v k w"))
        p4s = sb.tile([SH, SB], f32); nc.sync.dma_start(out=p4s[:], in_=p4s_d.ap())
        uw = sb.tile([wb, W], f32); nc.sync.dma_start(out=uw[:], in_=uw_d.ap())
        uhl = sb.tile([NO, SH], bf16); nc.sync.dma_start(out=uhl[:], in_=uh_d.ap())
        uhlf = sb.tile([NO, SH], f32); nc.sync.dma_start(out=uhlf[:], in_=uhf_d.ap())
        xwb = sb.tile([wb, C, wb], bf16)        # Up_w(msg-tmin) bf16, h padded to 128
        xtb = sb.tile([wb, 4, wb, C], bf16)     # xbar out: [h(34 valid), j, w, c]
        tcb = sb.tile([wb, wb], bf16)           # Up_w(tmin) bf16, h padded
        w01 = sb.tile([16, 32], f32)
        nc.sync.dma_start(out=w01[:, 0:16], in_=w0_d.ap())
        nc.sync.dma_start(out=w01[:, 16:32], in_=w1_d.ap())
        vcc = sb.tile([16, NH, wb], f32)        # gathered V, C-part
        rT = sb.tile([wb, 3, 46], f32)
        rhoT = sb.tile([wb, 46], f32)
        phi0 = sb.tile([wb, 46], f32)
        Dsum = sb.tile([SH, W], f32)
        Rrec = sb.tile([SH, W], f32)
        t8 = sb.tile([SH, 8, W], f32)
        t4 = sb.tile([SH, 4, W], f32)
        t2 = sb.tile([SH, 2, W], f32)

        vbounce = dr.tile([SB, C, wb], f32)
        gpad = dr.tile([140, C, wb], f32)
        v0d = dr.tile([C, NH, wb], f32)
        v1d = dr.tile([C, NH, wb], f32)
        xwd = dr.tile([4, wb, C, wb], bf16)
        twd = dr.tile([4, wb, wb], bf16)

        # ---------- init ----------
        with tc.tile_pool(name="ini", bufs=1) as ini:
            zpad = ini.tile([96, wb], f32)
            nc.vector.memset(zpad[:], 0.0)
            nc.sync.dma_start(out=gpad[:][0:6].rearrange("a b w -> (a b) w"), in_=zpad[:])
            nc.sync.dma_start(out=gpad[:][134:140].rearrange("a b w -> (a b) w"), in_=zpad[:])
            nc.vector.memset(xwb[:], 0.0)
            nc.vector.memset(tcb[:], 0.0)

            p4i = ini.tile([92, 23], f32)
            nc.sync.dma_start(out=p4i[:], in_=p4i_d.ap())
            for ch in range(2):
                imgc = ini.tile([92, 3, W], f32, tag="imgc")
                nc.sync.dma_start(
                    out=imgc[:],
                    in_=img_d.ap()[:, 92 * ch:92 * (ch + 1), :].rearrange("c h w -> h c w"))
                pw_ = ini.tile([92, 3, wb], f32, tag="pw_")
                nc.vector.reduce_sum(
                    out=pw_[:], in_=imgc[:].rearrange("p c (v k) -> p c v k", k=4), axis=X)
                with tc.tile_pool(name="psi", bufs=1, space="PSUM") as psi:
                    ip = psi.tile([23, 3, wb], f32, tag="ip")
                    nc.tensor.matmul(ip[:], p4i[:], pw_[:], start=True, stop=True)
                    ib = dr.tile([23, 3, wb], f32, tag="ib")
                    icp = ini.tile([23, 3, wb], f32, tag="icp")
                    nc.vector.tensor_copy(icp[:], ip[:])
                    nc.sync.dma_start(out=ib[:], in_=icp[:])
                for m3 in range(3):
                    nc.sync.dma_start(out=rT[:, m3, 23 * ch:23 * (ch + 1)],
                                      in_=ib[:][:, m3, :].rearrange("h w -> w h"))
            tmp3 = ini.tile([wb, 3, 46], f32)
            nc.vector.tensor_tensor(out=tmp3[:], in0=rT[:], in1=rT[:], op=AL.mult)
            nc.vector.reduce_sum(out=rhoT[:], in_=tmp3[:].rearrange("p m h -> p h m"), axis=X)
            nc.vector.tensor_scalar(out=phi0[:], in0=rhoT[:], scalar1=float(-C1 / 2.0),
                                    scalar2=float(C0), op0=AL.mult, op1=AL.add)

            # unary = softmax(x)
            nc.sync.dma_start(out=logq[:], in_=xs_d.ap().rearrange("c h w -> h c w"))
            nc.scalar.activation(out=logq[:], in_=logq[:], func=ACTF.Exp)
            nc.vector.tensor_tensor(out=t8[:], in0=logq[:][:, 0:8, :],
                                    in1=logq[:][:, 8:16, :], op=AL.add)
            nc.vector.tensor_tensor(out=t4[:], in0=t8[:][:, 0:4, :],
                                    in1=t8[:][:, 4:8, :], op=AL.add)
            nc.vector.tensor_tensor(out=t2[:], in0=t4[:][:, 0:2, :],
                                    in1=t4[:][:, 2:4, :], op=AL.add)
            nc.vector.tensor_tensor(out=Dsum[:], in0=t2[:][:, 0, :],
                                    in1=t2[:][:, 1, :], op=AL.add)
            nc.vector.reciprocal(out=Rrec[:], in_=Dsum[:])
            nc.vector.tensor_tensor(out=logq[:], in0=logq[:], in1=bc(Rrec[:], C), op=AL.mult)
            nc.vector.tensor_scalar(out=u08m[:], in0=logq[:], scalar1=UNARY_W,
                                    scalar2=UNARY_W, op0=AL.mult, op1=AL.subtract)
            nc.vector.tensor_scalar(out=logq[:], in0=logq[:], scalar1=1.0,
                                    scalar2=1.0, op0=AL.mult, op1=AL.subtract)

        # ---------- steps ----------
        for step in range(NUM_STEPS):
            last = step == NUM_STEPS - 1
            nc.scalar.activation(out=logq[:], in_=logq[:], func=ACTF.Exp)
            nc.vector.tensor_tensor(out=t8[:], in0=logq[:][:, 0:8, :],
                                    in1=logq[:][:, 8:16, :], op=AL.add)
            nc.vector.tensor_tensor(out=t4[:], in0=t8[:][:, 0:4, :],
                                    in1=t8[:][:, 4:8, :], op=AL.add)
            nc.vector.tensor_tensor(out=t2[:], in0=t4[:][:, 0:2, :],
                                    in1=t4[:][:, 2:4, :], op=AL.add)
            nc.vector.tensor_tensor(out=Dsum[:], in0=t2[:][:, 0, :],
                                    in1=t2[:][:, 1, :], op=AL.add)
            nc.vector.reciprocal(out=Rrec[:], in_=Dsum[:])
            nc.vector.tensor_tensor(out=logq[:], in0=logq[:], in1=bc(Rrec[:], C), op=AL.mult)
            qw = sc.tile([SH, C, wb], f32, tag="qw")
            nc.vector.reduce_sum(out=qw[:], in_=logq[:].rearrange("p c (v k) -> p c v k", k=4),
                                 axis=X)
            with tc.tile_pool(name="psv", bufs=1, space="PSUM") as psv:
                vps = psv.tile([SB, C, wb], f32, tag="vps")
                for g in range(4):           # chunk moving free to 512
                    nc.tensor.matmul(vps[:, 4 * g:4 * (g + 1), :], p4s[:],
                                     qw[:, 4 * g:4 * (g + 1), :], start=True, stop=True)
                vcp = sc.tile([SB, C, wb], f32, tag="cpy2")
                nc.vector.tensor_copy(vcp[:], vps[:])
                nc.sync.dma_start(out=vbounce[:], in_=vcp[:])
            nc.gpsimd.collective_compute(
                "AllGather", AL.bypass, replica_groups=[[0, 1, 2, 3], [4, 5, 6, 7]],
                ins=[vbounce[:].opt()], outs=[gpad[:][6:134].opt()])

            # compat (fp32), output directly w-on-partitions:
            # out[w, d] = sum_c V[c, h, w] * w01[c, d] per h row
            nc.scalar.dma_start(
                out=vcc[:, 6:38, :],
                in_=vbounce[:].rearrange("h c w -> c h w"))
            nc.sync.dma_start(
                out=vcc[:, 0:6, :],
                in_=gpad[:][bass.ds(q32, 6), :, :].rearrange("h c w -> c h w"))
            nc.sync.dma_start(
                out=vcc[:, 38:44, :],
                in_=gpad[:][bass.ds(q32 + 38, 6), :, :].rearrange("h c w -> c h w"))
            v0t = sc.tile([wb, C, NH], f32, tag="v0t")
            v1t = sc.tile([wb, C, NH], f32, tag="v1t")
            with tc.tile_pool(name="psc", bufs=1, space="PSUM") as psc:
                cpw = psc.tile([wb, NH, 32], f32, tag="cpw")
                for h in list(range(6, 38)) + list(range(0, 6)) + list(range(38, NH)):
                    nc.tensor.matmul(cpw[:, h, :], vcc[:, h, :], w01[:],
                                     start=True, stop=True)
                nc.vector.tensor_copy(
                    v0t[:], cpw[:][:, :, 0:16].rearrange("w h c -> w c h"))
                nc.vector.tensor_copy(
                    v1t[:], cpw[:][:, :, 16:32].rearrange("w h c -> w c h"))

            flds = []
            for m in range(3):
                f = sc.tile([wb, C, NH], f32, tag=f"fl{m}")
                nc.vector.tensor_tensor(out=f[:], in0=v0t[:], in1=bc(rT[:, m, 1:45], C),
                                        op=AL.mult)
                flds.append(f)
            f4 = sc.tile([wb, C, NH], f32, tag="fl4")
            nc.vector.tensor_tensor(out=f4[:], in0=v0t[:], in1=bc(rhoT[:, 1:45], C),
                                    op=AL.mult)

            msg = sc.tile([wb, C, NO], f32, tag="msg")
            tmpm = sc.tile([wb, 8, NO], f32, tag="tmpm")
            for cf in range(2):          # c-halves: psum + moving free <= 512
              with tc.tile_pool(name="psb", bufs=1, space="PSUM") as psb:
                cs = slice(8 * cf, 8 * (cf + 1))
                stiles = []
                for nm, srct, gdt in (("s0", v0t, gd0), ("s1", flds[0], gd0),
                                     ("s2", flds[1], gd0), ("s3", flds[2], gd0),
                                     ("s4", f4, gd0), ("sk", v1t, gd1)):
                    st = psb.tile([wb, 8, NO], f32, tag=nm)
                    # g1 beyond +-2 shifts is <= 7.2e-4: skip those taps
                    ks, ke = (3, 7) if gdt is gd1 else (0, KS - 1)
                    for k in range(ks, ke + 1):
                        nc.tensor.matmul(st[:], gdt[:, k, :], srct[:, cs, k:k + NO],
                                         start=(k == ks), stop=(k == ke))
                    stiles.append(st)
                s0, s1, s2, s3, s4, skt = stiles
                mh = msg[:, cs, :]
                nc.vector.tensor_tensor(out=mh, in0=s0[:], in1=bc(phi0[:, 6:6 + NO], 8),
                                        op=AL.mult)
                for m in range(3):
                    nc.vector.tensor_tensor(out=tmpm[:], in0=[s1, s2, s3][m][:],
                                            in1=bc(rT[:, m, 6:6 + NO], 8), op=AL.mult)
                    nc.vector.scalar_tensor_tensor(out=mh, in0=tmpm[:], scalar=float(C1),
                                                   in1=mh, op0=AL.mult, op1=AL.add)
                nc.vector.scalar_tensor_tensor(out=mh, in0=s4[:], scalar=float(-C1 / 2.0),
                                               in1=mh, op0=AL.mult, op1=AL.add)
                nc.vector.tensor_tensor(out=mh, in0=mh, in1=skt[:], op=AL.add)

            tmin = sc.tile([wb, NO], f32, tag="tmin")
            nc.vector.tensor_reduce(out=tmin[:], in_=msg[:].rearrange("p c h -> p h c"),
                                    axis=X, op=AL.min)
            nc.vector.tensor_tensor(out=msg[:], in0=msg[:], in1=bc(tmin[:], C),
                                    op=AL.subtract)

            for j in range(4):
              with tc.tile_pool(name="psu", bufs=1, space="PSUM") as psu:
                for cf in range(2):
                    pj = psu.tile([wb, 8, NO], f32, tag=f"pj{cf}")
                    nc.tensor.matmul(pj[:],
                                     uw[:, wb * j: wb * (j + 1)],
                                     msg[:, 8 * cf:8 * (cf + 1), :], start=True, stop=True)
                    nc.vector.tensor_copy(xwb[:, 8 * cf:8 * (cf + 1), 0:NO], pj[:])
                nc.sync.dma_start(out=xwd[:][j], in_=xwb[:])
                if last:
                    tj = psu.tile([wb, NO], f32, tag="tj")
                    nc.tensor.matmul(tj[:], uw[:, wb * j: wb * (j + 1)], tmin[:],
                                     start=True, stop=True)
                    nc.vector.tensor_copy(tcb[:, 0:NO], tj[:])
                    nc.sync.dma_start(out=twd[:][j], in_=tcb[:])

            nc.sync.dma_start_transpose(
                out=xtb[:][:, 0:2, :, :].rearrange("h j w c -> h (j w c)"),
                in_=xwd[:][0:2].rearrange("j w c h -> (j w c) h"))
            nc.scalar.dma_start_transpose(
                out=xtb[:][:, 2:4, :, :].rearrange("h j w c -> h (j w c)"),
                in_=xwd[:][2:4].rearrange("j w c h -> (j w c) h"))
            for half in range(2):
                with tc.tile_pool(name="psh", bufs=1, space="PSUM") as psh:
                    xp = psh.tile([SH, 8, W], f32, tag="xp")
                    for cc in range(8):
                        nc.tensor.matmul(
                            xp[:, cc, :], uhl[:],
                            xtb[0:NO, :, :, 8 * half + cc].rearrange(
                                "h j w -> h (j w)"),
                            start=True, stop=True)
                    nc.vector.scalar_tensor_tensor(
                        out=logq[:, 8 * half:8 * (half + 1), :], in0=xp[:], scalar=-1.0,
                        in1=u08m[:, 8 * half:8 * (half + 1), :], op0=AL.mult, op1=AL.add)
            if last:
                tt = sc.tile([wb, W], bf16, tag="tt")
                nc.sync.dma_start_transpose(
                    out=tt[:], in_=twd[:].rearrange("j w h -> (j w) h"))
                with tc.tile_pool(name="pst", bufs=1, space="PSUM") as pst:
                    tp = pst.tile([SH, W], f32, tag="tp")
                    nc.tensor.matmul(tp[:], uhl[:], tt[0:NO, :], start=True, stop=True)
                    upt = sc.tile([SH, W], f32, tag="upt")
                    nc.vector.tensor_scalar(out=upt[:], in0=tp[:], scalar1=-1.0,
                                            scalar2=UNARY_W, op0=AL.mult, op1=AL.add)
                nc.vector.tensor_tensor(out=logq[:], in0=logq[:], in1=bc(upt[:], C),
                                        op=AL.add)

        nc.sync.dma_start(out=out_d.ap().rearrange("c h w -> h c w"), in_=logq[:])

    nc.compile()
    return nc


def kernel(x, image, w_compat0, w_compat1):
    from concourse import bass_utils

    if "nc" not in _CACHE:
        _CACHE["consts"] = _host_consts()
        _CACHE["nc"] = _build()
    nc = _CACHE["nc"]
    cst = _CACHE["consts"]

    x = np.ascontiguousarray(x, np.float32)
    image = np.ascontiguousarray(image, np.float32)
    in_maps = []
    for cid in range(8):
        b, q = cid // 4, cid % 4
        r0 = 128 * q
        ie = np.zeros((3, 184, W), np.float32)
        lo, hi = r0 - 28, r0 + 156
        slo, shi = max(lo, 0), min(hi, H)
        ie[:, slo - lo:shi - lo, :] = image[b, :, slo:shi, :] / np.float32(RGB_SCALE)
        in_maps.append({
            "xs": np.ascontiguousarray(x[b, :, r0:r0 + 128, :]),
            "imge": ie,
            "uh": _bf16(cst["Uh_loc"][q]),
            "uhf": np.ascontiguousarray(cst["Uh_loc"][q]),
            "w0r": np.ascontiguousarray((PW0 * w_compat0).T.astype(np.float32)),
            "w1r": np.ascontiguousarray((PW1 * w_compat1).T.astype(np.float32)),
            "gd0": cst["Gd0"], "gd1": cst["Gd1"], "p4s": cst["P4s"],
            "p4i": cst["P4i"], "uw": cst["Uw"],
        })
    res = bass_utils.run_bass_kernel_spmd(nc, in_maps, core_ids=list(range(8)),
                                          **_CACHE.get("run_kwargs", {}))
    _CACHE["last_result"] = res
    out = np.empty((B, C, H, W), np.float32)
    for cid in range(8):
        b, q = cid // 4, cid % 4
        out[b, :, 128 * q:128 * (q + 1), :] = res.results[cid]["out"]
    return out

